# revision 1
# baseline (speedup 1.0000x reference)
"""Trainium2 Bass kernel for LoopRelationalGraphConvolution.

Math (matches the jax reference):
    out[n] = relu( SCALE * sum_s  W[rel[n,s]] @ emb[neighbors[n,s]] )
    SCALE  = 1000 / (R1 * S)      (folds the mean over S and the /R1 * 1000)

Design (8 NeuronCores, data-parallel over the 8192-node batch):
  Each core owns 1024 nodes, split into 9 node-tiles ([114]*8 + [112] nodes)
  chosen by a host-side balancer so that every (tile, relation) bucket has
  <=128 edges.  Per tile the device kernel:
    1. dma_gather(transpose=True): fetches the tile's 33*128 edge-slot
       embeddings (bf16, compacted per-core int16 ids) with the embedding dim
       landing on SBUF partitions:  ET[p, c, i] = emb[idx_i][c*128+p].
    2. stage-1 matmuls: per relation-chunk r (128 slots), in 2 K-chunks over D:
       Y[slot, o] += ET[:, c, slots]^T @ W_r[c]       (PSUM, f32)
    3. stage-2 matmul: 0/1 selection matrix reduces edge slots into node rows:
       out_psum[node, o] += SEL_r^T @ Y_bf16          (accumulated over all r)
    4. relu on PSUM->SBUF evacuation, DMA node rows to DRAM.
  The device program is fully static and identical across cores (SPMD); all
  data-dependence lives in the index / selection arrays.  Host post-step
  inverse-permutes rows back to the original node order.
"""

import numpy as np
import ml_dtypes

bf16 = ml_dtypes.bfloat16
fp8 = ml_dtypes.float8_e4m3

# Problem constants (hardcoded per contract).
V = 100000
D = 256
R1 = 33          # relations incl. self-loop
N = 8192
S = 32
NCORES = 8
NPC = N // NCORES          # 1024 nodes per core
NTILES = 9                 # node-tiles per core
CAPS = [114] * 8 + [112]   # nodes per tile (uniform across cores)
ROW_BASE = np.concatenate([[0], np.cumsum(CAPS)]).tolist()
P = 128
NSLOT = R1 * P             # 4224 edge slots per tile
GSPLIT = [0, 2, 6, 12, 19, 26, R1]   # gather segment chunk boundaries
GSEG = [(a * P, b * P) for a, b in zip(GSPLIT, GSPLIT[1:])]
IDXW = NSLOT // 16         # 264 int16 idx columns (16-partition wrap)
UMAX = 32768               # compacted per-core embedding rows (int16 limit)
SCALE = 1000.0 / (R1 * S)

# Software-pipeline skew between stage-1 and stage-2 of consecutive chunks,
# so the PE never stalls on the PSUM->SBUF copy of the current chunk.
SKEW = 6
PF = 3      # tile prefetch depth


# ---------------------------------------------------------------------------
# Host-side preparation
# ---------------------------------------------------------------------------

def _balance_tiles(hist):
    """Assign NPC nodes to NTILES tiles (exactly CAPS[t] nodes each),
    minimizing the max per-(tile, relation) edge count. hist: [NPC, R1].
    Greedy: hardest nodes first, place on the tile minimizing the resulting
    peak bucket."""
    order = np.argsort(-hist.max(axis=1), kind="stable")
    loads = np.zeros((NTILES, R1), dtype=np.int64)
    counts = np.zeros(NTILES, dtype=np.int64)
    tiles = [[] for _ in range(NTILES)]
    for n in order:
        h = hist[n]
        best_t, best_key = -1, None
        for t in range(NTILES):
            if counts[t] >= CAPS[t]:
                continue
            new = loads[t] + h
            key = (int(new.max()), int(loads[t].max()), int(new.sum()))
            if best_key is None or key < best_key:
                best_key, best_t = key, t
        tiles[best_t].append(int(n))
        loads[best_t] += h
        counts[best_t] += 1
    return tiles, loads


def prep(emb_table, weights, neighbors, relations):
    """Build per-core device arrays. Returns (in_maps, perms)."""
    emb_bf = np.asarray(emb_table).astype(bf16)
    w = np.asarray(weights, dtype=np.float32) * SCALE         # [R1, D_out, D_in]
    # W_sb[p, (r*2 + c)*D + o] = w[r, o, c*128+p]
    w_rdo = np.ascontiguousarray(w.transpose(0, 2, 1))        # [r, d, o]
    W_sb = np.ascontiguousarray(
        w_rdo.reshape(R1, 2, 128, D).transpose(2, 0, 1, 3)    # [p, r, c, o]
    ).reshape(128, R1 * 2 * D).astype(bf16)

    neighbors = np.asarray(neighbors).astype(np.int64)
    relations = np.asarray(relations).astype(np.int64)

    in_maps, perms = [], []
    for c in range(NCORES):
        nb = neighbors[c * NPC:(c + 1) * NPC]                 # [NPC, S]
        rel = relations[c * NPC:(c + 1) * NPC]
        uniq, inv = np.unique(nb.ravel(), return_inverse=True)
        inv = inv.reshape(nb.shape).astype(np.int64)
        U = len(uniq)
        assert U <= UMAX, U
        emb_c = np.zeros((UMAX, D), dtype=bf16)
        emb_c[:U] = emb_bf[uniq]

        hist = np.zeros((NPC, R1), dtype=np.int64)
        np.add.at(hist, (np.repeat(np.arange(NPC), S), rel.ravel()), 1)
        tiles, loads = _balance_tiles(hist)
        assert loads.max() <= P, f"balance failed: max bucket {loads.max()}"

        idx_all = np.zeros((NTILES, 128, IDXW), dtype=np.int16)
        sel_all = np.zeros((NTILES, 128, NSLOT), dtype=fp8)
        perm = []
        for t, nodes in enumerate(tiles):
            nodes = np.array(nodes, dtype=np.int64)
            ncnt = len(nodes)
            assert ncnt == CAPS[t]
            perm.extend((c * NPC + nodes).tolist())
            # edges of this tile
            er = rel[nodes].ravel()                            # relation per edge
            ei = inv[nodes].ravel()                            # compact nbr id
            ej = np.repeat(np.arange(ncnt), S)                 # local node idx
            order = np.argsort(er, kind="stable")
            er_s, ei_s, ej_s = er[order], ei[order], ej[order]
            # position within relation group
            start = np.searchsorted(er_s, np.arange(R1))
            pos = np.arange(ncnt * S) - start[er_s]
            slot = er_s * P + pos                              # [ncnt*S]
            slots_idx = np.zeros(NSLOT, dtype=np.int16)
            slots_idx[slot] = ei_s
            sel = np.zeros((NSLOT, 128), dtype=fp8)
            sel[slot, ej_s] = 1.0
            # idx wrap per gather segment: idx i at partition i%16, col i//16
            wrapped = np.concatenate(
                [slots_idx[a:b].reshape((b - a) // 16, 16).T
                 for a, b in GSEG], axis=1)                    # [16, IDXW]
            idx_all[t] = np.tile(wrapped, (8, 1))
            # device SEL layout: [part p = slot-in-chunk, free = r*128 + node]
            sel_all[t] = np.ascontiguousarray(
                sel.reshape(R1, P, 128).transpose(1, 0, 2).reshape(P, NSLOT))
        in_maps.append({
            "emb": emb_c,
            "wsb": W_sb,
            "idx": np.ascontiguousarray(idx_all.reshape(NTILES * 128, IDXW)),
            "sel": np.ascontiguousarray(sel_all.reshape(NTILES * 128, NSLOT)),
        })
        perms.append(np.array(perm, dtype=np.int64))

    return in_maps, perms


# ---------------------------------------------------------------------------
# Numpy emulation (bf16-faithful) for validation
# ---------------------------------------------------------------------------

def emulate_core(in_map):
    emb = in_map["emb"]                                        # [UMAX, D] bf16
    wsb = in_map["wsb"].reshape(128, R1, 2, D)                 # [p, r, c, o]
    idx = in_map["idx"].reshape(NTILES, 128, IDXW)
    sel = in_map["sel"].reshape(NTILES, 128, NSLOT)
    out = np.zeros((NPC, D), dtype=np.float32)
    for t in range(NTILES):
        parts, col = [], 0
        for a, b in GSEG:
            w = (b - a) // 16
            parts.append(idx[t, :16, col:col + w].T.reshape(b - a))
            col += w
        slots_idx = np.concatenate(parts)                      # unwrap
        X = emb[slots_idx]                                     # [NSLOT, D] bf16
        out_acc = np.zeros((128, D), dtype=np.float32)
        for r in range(R1):
            Xr = X[r * P:(r + 1) * P].astype(np.float32)       # [128, D]
            Y = (Xr[:, :128] @ wsb[:, r, 0, :].astype(np.float32)
                 + Xr[:, 128:] @ wsb[:, r, 1, :].astype(np.float32))
            Yb = Y.astype(bf16).astype(np.float32)             # PSUM->SBUF bf16
            selr = sel[t][:, r * 128:(r + 1) * 128].astype(np.float32)
            out_acc += selr.T @ Yb
        base, ncnt = ROW_BASE[t], CAPS[t]
        out[base:base + ncnt] = np.maximum(out_acc[:ncnt], 0.0)
    return out


def emulate(emb_table, weights, neighbors, relations):
    in_maps, perms = prep(emb_table, weights, neighbors, relations)
    full = np.zeros((N, D), dtype=np.float32)
    for c in range(NCORES):
        full[perms[c]] = emulate_core(in_maps[c])
    return full


# ---------------------------------------------------------------------------
# Bass program
# ---------------------------------------------------------------------------

def build_program():
    import concourse.bacc as bacc
    import concourse.tile as tile
    import concourse.mybir as mybir

    nc = bacc.Bacc(
        "TRN2", target_bir_lowering=False, debug=False,
        num_devices=NCORES,
    )
    BF = mybir.dt.bfloat16
    F32 = mybir.dt.float32
    I16 = mybir.dt.int16
    F8 = mybir.dt.float8e4

    emb = nc.dram_tensor("emb", [UMAX, D], BF, kind="ExternalInput").ap()
    wsb = nc.dram_tensor("wsb", [128, R1 * 2 * D], BF, kind="ExternalInput").ap()
    idx = nc.dram_tensor("idx", [NTILES * 128, IDXW], I16, kind="ExternalInput").ap()
    sel = nc.dram_tensor("sel", [NTILES * 128, NSLOT], F8,
                         kind="ExternalInput").ap()
    out = nc.dram_tensor("out", [NPC, D], F32, kind="ExternalOutput").ap()

    Relu = mybir.ActivationFunctionType.Relu

    with tile.TileContext(nc) as tc:
        with (
            tc.tile_pool(name="wpool", bufs=1) as wpool,
            tc.tile_pool(name="etpool", bufs=PF + 1) as etpool,
            tc.tile_pool(name="selpool", bufs=PF + 1) as selpool,
            tc.tile_pool(name="idxpool", bufs=PF + 1) as idxpool,
            tc.tile_pool(name="ypool", bufs=2 * (SKEW + 1)) as ypool,
            tc.tile_pool(name="opool", bufs=2) as opool,
            tc.tile_pool(name="psy", bufs=SKEW + 1, space="PSUM") as psy,
            tc.tile_pool(name="pso", bufs=1, space="PSUM") as pso,
        ):
            wt = wpool.tile([128, R1 * 2 * D], BF)

            def load_w(r0, r1):
                for r in range(r0, r1):
                    a, b = r * 2 * D, (r + 1) * 2 * D
                    nc.sync.dma_start(out=wt[:, a:b], in_=wsb[:, a:b])

            ets, sels = {}, {}

            def pre_gather(t):
                idx_t = idxpool.tile([128, IDXW], I16, name="idx_t")
                nc.sync.dma_start(
                    out=idx_t[:], in_=idx[t * 128:(t + 1) * 128, :])
                segs, col = [], 0
                for gi, (a, b) in enumerate(GSEG):
                    n = b - a
                    eth = etpool.tile([128, 2, n], BF, name=f"et{gi}")
                    nc.gpsimd.dma_gather(
                        out_ap=eth[:],
                        in_ap=emb,
                        idxs_ap=idx_t[:, col:col + n // 16],
                        num_idxs=n,
                        num_idxs_reg=n,
                        elem_size=D,
                        transpose=True,
                        single_packet=False,
                    )
                    col += n // 16
                    segs.append(eth)
                ets[t] = segs

            def pre_sel(t):
                sel_t = selpool.tile([128, NSLOT], F8, name="sel_t")
                nc.sync.dma_start(
                    out=sel_t[:], in_=sel[t * 128:(t + 1) * 128, :])
                sels[t] = sel_t

            def prefetch(t):
                if t >= NTILES:
                    return
                pre_gather(t)
                pre_sel(t)

            # startup orchestration: gather(0) first, early W chunks, sel(0),
            # then stream the rest so the PE can start by ~9us.
            pre_gather(0)
            load_w(0, 11)
            pre_sel(0)
            pre_gather(1)
            load_w(11, R1)
            pre_sel(1)
            prefetch(2)

            for t in range(NTILES):
                segs, sel_t = ets.pop(t), sels.pop(t)
                outp = pso.tile([128, D], F32)
                ys = [None] * R1
                for r in range(R1 + SKEW):
                    if r < R1:
                        yp = psy.tile([128, D], F32, name="yp")
                        gi = next(i for i, (a, b) in enumerate(GSEG)
                                  if a <= r * P < b)
                        eth, off = segs[gi], r * P - GSEG[gi][0]
                        for c in range(2):
                            nc.tensor.matmul(
                                out=yp[:],
                                lhsT=eth[:, c, off:off + P],
                                rhs=wt[:, (r * 2 + c) * D:(r * 2 + c + 1) * D],
                                start=(c == 0), stop=(c == 1),
                            )
                        ysb = ypool.tile([128, D], BF, name="ysb")
                        if r % 2 == 0:
                            nc.vector.tensor_copy(out=ysb[:], in_=yp[:])
                        else:
                            nc.scalar.copy(out=ysb[:], in_=yp[:])
                        ys[r] = ysb
                    if r >= SKEW:
                        q = r - SKEW
                        nc.tensor.matmul(
                            out=outp[:],
                            lhsT=sel_t[:, q * P:(q + 1) * P],
                            rhs=ys[q][:],
                            start=(q == 0), stop=(q == R1 - 1),
                        )
                prefetch(t + PF)
                osb = opool.tile([128, D], F32)
                nc.scalar.activation(out=osb[:], in_=outp[:], func=Relu)
                base, ncnt = ROW_BASE[t], CAPS[t]
                nc.sync.dma_start(
                    out=out[base:base + ncnt, :], in_=osb[:ncnt, :])

    nc.compile()
    return nc


_NC_CACHE = []


def _get_program():
    if not _NC_CACHE:
        _NC_CACHE.append(build_program())
    return _NC_CACHE[0]


# ---------------------------------------------------------------------------
# Entry point
# ---------------------------------------------------------------------------

def kernel(emb_table, weights, neighbors, relations):
    from concourse import bass_utils

    in_maps, perms = prep(emb_table, weights, neighbors, relations)
    nc = _get_program()
    res = bass_utils.run_bass_kernel_spmd(
        nc, in_maps, core_ids=list(range(NCORES)),
    )
    full = np.zeros((N, D), dtype=np.float32)
    for c in range(NCORES):
        full[perms[c]] = res.results[c]["out"]
    return full



# revision 20
# speedup vs baseline: 1.1529x; 1.1529x over previous
"""Trainium2 Bass kernel for LoopRelationalGraphConvolution.

Math (matches the jax reference):
    out[n] = relu( SCALE * sum_s  W[rel[n,s]] @ emb[neighbors[n,s]] )
    SCALE  = 1000 / (R1 * S)      (folds the mean over S and the /R1 * 1000)

Design (8 NeuronCores, data-parallel over the 8192-node batch):
  Each core owns 1024 nodes, split into 9 node-tiles ([114]*8 + [112] nodes)
  chosen by a host-side balancer so that every (tile, relation) bucket has
  <=128 edges.  Embeddings and weights are carried as fp8 hi/lo pairs so the
  stage-1 matmuls can run in fp8 DoubleRow mode (K=256 per instruction at
  0.5 cycles/row = 4x the bf16 row rate):
      emb row (DRAM, 512B):  [Eh = fp8(e) | El = fp8(e - Eh)]
      weights (SBUF):        Wh = fp8(32*SCALE*w),  Wl = fp8(32*SCALE*w - Wh)
  Per tile the device kernel:
    1. dma_gather(transpose=True): one gather per tile fetches the 33*128
       edge-slot rows; the 16-bit transpose granularity lands bytes as
       ET[p, c, i, b] = row_i[2*(c*128+p)+b], i.e. the (hi|lo) halves become
       c=0/c=1 and the DoubleRow pair index b carries d=2p+b.
    2. stage-1, per relation r (128 slots), 3 DoubleRow matmuls into one PSUM:
       P = Eh^T Wh + El^T Wh + Eh^T Wl  ~=  32*SCALE * emb^T w   (f32)
    3. stage-2 matmul: selection matrix (entries 1/32, folding the scale)
       reduces edge slots into node rows: out_psum[node, o] += SEL_r^T P_bf16.
    4. relu on PSUM->SBUF evacuation, DMA node rows to DRAM.
  The device program is fully static and identical across cores (SPMD); all
  data-dependence lives in the index / selection arrays.  Host post-step
  inverse-permutes rows back to the original node order.
"""

import numpy as np
import ml_dtypes

bf16 = ml_dtypes.bfloat16
fp8 = ml_dtypes.float8_e4m3

# Problem constants (hardcoded per contract).
V = 100000
D = 256
R1 = 33          # relations incl. self-loop
N = 8192
S = 32
NCORES = 8
NPC = N // NCORES          # 1024 nodes per core
NTILES = 9                 # node-tiles per core
CAPS = [114] * 8 + [112]   # nodes per tile (uniform across cores)
ROW_BASE = np.concatenate([[0], np.cumsum(CAPS)]).tolist()
P = 128
NSLOT = R1 * P             # 4224 edge slots per tile
IDXW = NSLOT // 16         # 264 int16 idx columns (16-partition wrap)
UMAX = 32768               # compacted per-core embedding rows (int16 limit)
SCALE = 1000.0 / (R1 * S)
WSC = 32.0                 # extra weight scale so Wl stays in fp8-normal range

# Software-pipeline skew between stage-1 and stage-2 of consecutive chunks,
# so the PE never stalls on the PSUM->SBUF copy of the current chunk.
SKEW = 6
PF = 2      # tile prefetch depth


# ---------------------------------------------------------------------------
# Host-side preparation
# ---------------------------------------------------------------------------

def _balance_tiles(hist):
    """Assign NPC nodes to NTILES tiles (exactly CAPS[t] nodes each),
    minimizing the max per-(tile, relation) edge count. hist: [NPC, R1].
    Greedy: hardest nodes first, place on the tile minimizing the resulting
    peak bucket."""
    order = np.argsort(-hist.max(axis=1), kind="stable")
    loads = np.zeros((NTILES, R1), dtype=np.int64)
    counts = np.zeros(NTILES, dtype=np.int64)
    tiles = [[] for _ in range(NTILES)]
    for n in order:
        h = hist[n]
        best_t, best_key = -1, None
        for t in range(NTILES):
            if counts[t] >= CAPS[t]:
                continue
            new = loads[t] + h
            key = (int(new.max()), int(loads[t].max()), int(new.sum()))
            if best_key is None or key < best_key:
                best_key, best_t = key, t
        tiles[best_t].append(int(n))
        loads[best_t] += h
        counts[best_t] += 1
    return tiles, loads


def _split_hi_lo(x):
    """fp8 hi/lo decomposition: x ~= hi + lo with hi = fp8(x), lo = fp8(x-hi)."""
    hi = x.astype(fp8)
    lo = (x - hi.astype(np.float32)).astype(fp8)
    return hi, lo


def prep(emb_table, weights, neighbors, relations):
    """Build per-core device arrays. Returns (in_maps, perms)."""
    emb_f32 = np.asarray(emb_table, dtype=np.float32)
    eh, el = _split_hi_lo(emb_f32)                            # [V, D] fp8 each
    # row byte 2u = Eh[u], byte 2u+1 = El[u]: the gather's 16-bit transpose
    # granularity then lands Eh at b=0 and El at b=1 with d = c*128+p.
    emb_pair = np.empty((emb_f32.shape[0], 2 * D), dtype=fp8)
    emb_pair.view(np.uint8)[:, 0::2] = eh.view(np.uint8)
    emb_pair.view(np.uint8)[:, 1::2] = el.view(np.uint8)

    w32 = np.asarray(weights, dtype=np.float32) * (SCALE * WSC)  # [R1, Do, Di]
    wh, wl = _split_hi_lo(np.ascontiguousarray(w32.transpose(0, 2, 1)))  # [r,d,o]
    # W_sb[p, r, m, i, o] = w_m[r, i*128+p, o]  (m: 0=hi, 1=lo; i: DoubleRow pair)
    W_sb = np.zeros((128, R1, 2, 2, D), dtype=fp8)
    for i in range(2):
        W_sb[:, :, 0, i, :] = wh[:, i * 128:(i + 1) * 128, :].transpose(1, 0, 2)
        W_sb[:, :, 1, i, :] = wl[:, i * 128:(i + 1) * 128, :].transpose(1, 0, 2)
    W_sb = np.ascontiguousarray(W_sb.reshape(128, R1 * 4 * D))

    neighbors = np.asarray(neighbors).astype(np.int64)
    relations = np.asarray(relations).astype(np.int64)

    in_maps, perms = [], []
    for c in range(NCORES):
        nb = neighbors[c * NPC:(c + 1) * NPC]                 # [NPC, S]
        rel = relations[c * NPC:(c + 1) * NPC]
        uniq, inv = np.unique(nb.ravel(), return_inverse=True)
        inv = inv.reshape(nb.shape).astype(np.int64)
        U = len(uniq)
        assert U <= UMAX, U
        emb_c = np.zeros((UMAX, 2 * D), dtype=fp8)
        emb_c[:U] = emb_pair[uniq]
        emb_c = emb_c.view(bf16)                          # [UMAX, D] bf16 view

        hist = np.zeros((NPC, R1), dtype=np.int64)
        np.add.at(hist, (np.repeat(np.arange(NPC), S), rel.ravel()), 1)
        tiles, loads = _balance_tiles(hist)
        assert loads.max() <= P, f"balance failed: max bucket {loads.max()}"

        idx_all = np.zeros((NTILES, 128, IDXW), dtype=np.int16)
        sel_all = np.zeros((NTILES, 128, NSLOT), dtype=fp8)
        perm = []
        for t, nodes in enumerate(tiles):
            nodes = np.array(nodes, dtype=np.int64)
            ncnt = len(nodes)
            assert ncnt == CAPS[t]
            perm.extend((c * NPC + nodes).tolist())
            # edges of this tile
            er = rel[nodes].ravel()                            # relation per edge
            ei = inv[nodes].ravel()                            # compact nbr id
            ej = np.repeat(np.arange(ncnt), S)                 # local node idx
            order = np.argsort(er, kind="stable")
            er_s, ei_s, ej_s = er[order], ei[order], ej[order]
            # position within relation group
            start = np.searchsorted(er_s, np.arange(R1))
            pos = np.arange(ncnt * S) - start[er_s]
            slot = er_s * P + pos                              # [ncnt*S]
            slots_idx = np.zeros(NSLOT, dtype=np.int16)
            slots_idx[slot] = ei_s
            sel = np.zeros((NSLOT, 128), dtype=fp8)
            sel[slot, ej_s] = 1.0 / WSC
            # idx wrap: idx i at partition i%16, col i//16 (one gather per tile)
            wrapped = slots_idx.reshape(IDXW, 16).T            # [16, IDXW]
            idx_all[t] = np.tile(wrapped, (8, 1))
            # device SEL layout: [part p = slot-in-chunk, free = r*128 + node]
            sel_all[t] = np.ascontiguousarray(
                sel.reshape(R1, P, 128).transpose(1, 0, 2).reshape(P, NSLOT))
        in_maps.append({
            "emb": emb_c,
            "wsb": W_sb,
            "idx": np.ascontiguousarray(idx_all.reshape(NTILES * 128, IDXW)),
            "sel": np.ascontiguousarray(sel_all.reshape(NTILES * 128, NSLOT)),
        })
        perms.append(np.array(perm, dtype=np.int64))

    return in_maps, perms


# ---------------------------------------------------------------------------
# Numpy emulation (fp8/bf16-faithful) for validation
# ---------------------------------------------------------------------------

def emulate_core(in_map):
    emb = in_map["emb"]                                        # [UMAX, 2D] fp8
    wsb = in_map["wsb"].reshape(128, R1, 2, 2, D)              # [p, r, m, i, o]
    idx = in_map["idx"].reshape(NTILES, 128, IDXW)
    sel = in_map["sel"].reshape(NTILES, 128, NSLOT)
    # reconstruct w_m[r, d, o] from the pair layout (d = i*128+p)
    w_hi = wsb[:, :, 0].astype(np.float32).transpose(1, 2, 0, 3).reshape(R1, D, D)
    w_lo = wsb[:, :, 1].astype(np.float32).transpose(1, 2, 0, 3).reshape(R1, D, D)
    out = np.zeros((NPC, D), dtype=np.float32)
    for t in range(NTILES):
        slots_idx = idx[t, :16, :].T.reshape(NSLOT)            # unwrap
        X = emb[slots_idx].astype(np.float32)                  # [NSLOT, 2D]
        Xh, Xl = X[:, 0::2], X[:, 1::2]
        out_acc = np.zeros((128, D), dtype=np.float32)
        for r in range(R1):
            sh = slice(r * P, (r + 1) * P)
            Yp = (Xh[sh] @ w_hi[r] + Xl[sh] @ w_hi[r] + Xh[sh] @ w_lo[r])
            Yb = Yp.astype(bf16).astype(np.float32)            # PSUM->SBUF bf16
            selr = sel[t][:, r * 128:(r + 1) * 128].astype(np.float32)
            out_acc += selr.T @ Yb
        base, ncnt = ROW_BASE[t], CAPS[t]
        out[base:base + ncnt] = np.maximum(out_acc[:ncnt], 0.0)
    return out


def emulate(emb_table, weights, neighbors, relations):
    in_maps, perms = prep(emb_table, weights, neighbors, relations)
    full = np.zeros((N, D), dtype=np.float32)
    for c in range(NCORES):
        full[perms[c]] = emulate_core(in_maps[c])
    return full


# ---------------------------------------------------------------------------
# Bass program
# ---------------------------------------------------------------------------

def build_program():
    import concourse.bacc as bacc
    import concourse.tile as tile
    import concourse.mybir as mybir

    nc = bacc.Bacc(
        "TRN2", target_bir_lowering=False, debug=False,
        num_devices=NCORES,
    )
    BF = mybir.dt.bfloat16
    F32 = mybir.dt.float32
    I16 = mybir.dt.int16
    F8 = mybir.dt.float8e4

    emb = nc.dram_tensor("emb", [UMAX, D], BF, kind="ExternalInput").ap()
    wsb = nc.dram_tensor("wsb", [128, R1 * 4 * D], F8, kind="ExternalInput").ap()
    idx = nc.dram_tensor("idx", [NTILES * 128, IDXW], I16, kind="ExternalInput").ap()
    sel = nc.dram_tensor("sel", [NTILES * 128, NSLOT], F8,
                         kind="ExternalInput").ap()
    out = nc.dram_tensor("out", [NPC, D], BF, kind="ExternalOutput").ap()

    Relu = mybir.ActivationFunctionType.Relu
    DR = mybir.MatmulPerfMode.DoubleRow

    with tile.TileContext(nc) as tc:
        with (
            tc.tile_pool(name="wpool", bufs=1) as wpool,
            tc.tile_pool(name="etpool", bufs=PF + 4) as etpool,
            tc.tile_pool(name="selpool", bufs=PF + 1) as selpool,
            tc.tile_pool(name="idxpool", bufs=PF + 1) as idxpool,
            tc.tile_pool(name="ypool", bufs=2 * (SKEW + 1)) as ypool,
            tc.tile_pool(name="opool", bufs=2) as opool,
            tc.tile_pool(name="psy", bufs=SKEW + 1, space="PSUM") as psy,
            tc.tile_pool(name="pso", bufs=1, space="PSUM") as pso,
        ):
            # W_sb[p, r, m, b, o]
            wt = wpool.tile([128, R1, 2, 2, D], F8)

            def load_w(r0, r1):
                nc.sync.dma_start(
                    out=wt[:, r0:r1],
                    in_=wsb[:, r0 * 4 * D:r1 * 4 * D])

            ets, sels = {}, {}

            def pre_gather(t, segs=(NSLOT,)):
                idx_t = idxpool.tile([128, IDXW], I16, name="idx_t")
                nc.sync.dma_start(
                    out=idx_t[:], in_=idx[t * 128:(t + 1) * 128, :])
                # bf16 unit (c*128+p) of row i -> ET[p, c, i]; its two bytes
                # are (Eh[c*128+p], El[c*128+p]) of slot i's embedding.
                parts, base = [], 0
                for n in segs:
                    eth = etpool.tile([128, 2, n], BF, name="et")
                    nc.gpsimd.dma_gather(
                        out_ap=eth[:],
                        in_ap=emb,
                        idxs_ap=idx_t[:, base // 16:(base + n) // 16],
                        num_idxs=n,
                        num_idxs_reg=n,
                        elem_size=D,
                        transpose=True,
                        single_packet=False,
                    )
                    parts.append((base, n, eth))
                    base += n
                ets[t] = parts

            def pre_sel(t):
                sel_t = selpool.tile([128, NSLOT], F8, name="sel_t")
                nc.sync.dma_start(
                    out=sel_t[:], in_=sel[t * 128:(t + 1) * 128, :])
                sels[t] = sel_t

            def prefetch(t):
                if t >= NTILES or t <= 2:
                    return
                pre_gather(t)
                pre_sel(t)

            # Startup orchestration. The W load (12us of DMA) is the startup
            # debt; it is sliced into small chunks interleaved with the first
            # gathers (FIFO DMA queue) so relation r's weights land just
            # before tile 0's stage-1 consumes them. Tiles 0/1 gather in
            # segments so the PE can start after the first ~1k slots.
            pre_gather(0, segs=(1024, 1536, 1664))
            load_w(0, 2)
            pre_sel(0)
            load_w(2, 5)
            load_w(5, 9)
            pre_gather(1, segs=(2048, 2176))
            load_w(9, 13)
            load_w(13, 17)
            pre_sel(1)
            load_w(17, 21)
            load_w(21, 25)
            pre_gather(2)
            load_w(25, 29)
            load_w(29, R1)
            pre_sel(2)

            for t in range(NTILES):
                parts, sel_t = ets.pop(t), sels.pop(t)
                et8s = [(base, n,
                         eth[:].bitcast(F8).rearrange("p c (s b) -> p c s b",
                                                      b=2))
                        for base, n, eth in parts]
                outp = pso.tile([128, D], F32)
                ys = [None] * R1
                for r in range(R1 + SKEW):
                    if r < R1:
                        yp = psy.tile([128, D], F32, name="yp")
                        base, n, et8 = next(
                            s for s in et8s
                            if s[0] <= r * P < s[0] + s[1])
                        sh = slice(r * P - base, (r + 1) * P - base)
                        # lhsT [p, i(2), slot]: b=0 -> Eh, b=1 -> El
                        ehT = et8[:, :, sh, 0]
                        elT = et8[:, :, sh, 1]
                        nc.tensor.matmul(
                            out=yp[:], lhsT=ehT, rhs=wt[:, r, 0],
                            start=True, stop=False, perf_mode=DR)
                        nc.tensor.matmul(
                            out=yp[:], lhsT=elT, rhs=wt[:, r, 0],
                            start=False, stop=False, perf_mode=DR)
                        nc.tensor.matmul(
                            out=yp[:], lhsT=ehT, rhs=wt[:, r, 1],
                            start=False, stop=True, perf_mode=DR)
                        ysb = ypool.tile([128, D], BF, name="ysb")
                        if r % 2 == 0:
                            nc.vector.tensor_copy(out=ysb[:], in_=yp[:])
                        else:
                            nc.scalar.copy(out=ysb[:], in_=yp[:])
                        ys[r] = ysb
                    if r >= SKEW:
                        q = r - SKEW
                        nc.tensor.matmul(
                            out=outp[:],
                            lhsT=sel_t[:, q * P:(q + 1) * P],
                            rhs=ys[q][:],
                            start=(q == 0), stop=(q == R1 - 1),
                        )
                prefetch(t + PF)
                osb = opool.tile([128, D], BF)
                nc.scalar.activation(out=osb[:], in_=outp[:], func=Relu)
                base, ncnt = ROW_BASE[t], CAPS[t]
                nc.sync.dma_start(
                    out=out[base:base + ncnt, :], in_=osb[:ncnt, :])

    nc.compile()
    return nc


_NC_CACHE = []


def _get_program():
    if not _NC_CACHE:
        _NC_CACHE.append(build_program())
    return _NC_CACHE[0]


# ---------------------------------------------------------------------------
# Entry point
# ---------------------------------------------------------------------------

def kernel(emb_table, weights, neighbors, relations):
    from concourse import bass_utils

    in_maps, perms = prep(emb_table, weights, neighbors, relations)
    nc = _get_program()
    res = bass_utils.run_bass_kernel_spmd(
        nc, in_maps, core_ids=list(range(NCORES)),
    )
    full = np.zeros((N, D), dtype=np.float32)
    for c in range(NCORES):
        full[perms[c]] = res.results[c]["out"].astype(np.float32)
    return full
